# revision 8
# baseline (speedup 1.0000x reference)
"""Fused single-head CNN self-attention kernel for Trainium2 (8 NeuronCores).

Computes, per batch b:
    q = (Wq/sqrt(C)) @ x + bq/sqrt(C)   (Cqk=32, N=4096)
    k = Wk @ x + bk
    v = Wv @ x + bv
    E[i, j]  = q[:, i] . k[:, j]        (already scaled by 1/sqrt(C))
    P        = softmax_j(E)
    out[c,i] = gamma * sum_j P[i, j] v[c, j] + x[c, i]

Sharding: B=4 batches x 2 query-halves -> 8 cores, no cross-core comms.
Each core handles one batch's full keys/values and 2048 queries.

Measured TRN2 facts this kernel is built around:
  * The PE does 16384 MACs/cycle regardless of dtype: fp8 gives NO matmul
    throughput (DoubleRow only packs 2x contraction per instruction).
    So E / P@V stay bf16; the MAC floor is the AV chain (~55us/core).
  * Projections run as fp8 DoubleRow matmuls anyway: same speed, but the
    x input DMA halves to 1MB and instruction count drops.
  * ACT exp runs at 1 elem/cycle/lane @1.2GHz (dtype-independent): the
    65536 exp-columns/core would cost ~68us on ACT alone - more than the
    PE strip time - so exp is split between ACT (true Exp) and DVE
    (Schraudolph bit-trick: int16(E*128*log2e + beta) bitcast to bf16,
    zero-mean ~0.25% sawtooth). GPSIMD cannot read PSUM (only other lane).
  * E^T tiles live in a 4-bank PSUM arena (block b -> bank b%4) so E
    matmuls, exp consumers, and AV overlap without pool coupling.
  * Normalize + residual fuse into one DVE scalar_tensor_tensor
    (pso * recip(denom) + x^T), with x^T prefetched in fp16 and the
    output stored fp16 (host upconverts) - no fp32 residual DMA traffic.
"""

import os

import numpy as np
import ml_dtypes

import concourse.bass as bass
import concourse.mybir as mybir
from concourse import bacc
from concourse.tile import TileContext
from concourse.bass_utils import run_bass_kernel_spmd

# Problem shape (hardcoded per contest contract).
B, C, H, W = 4, 256, 64, 64
N = H * W          # 4096 keys per batch
D = 32             # q/k head dim
NCORES = 8
MQ = N // 2        # 2048 queries per core
MQ_CHUNK = 512     # query strip width (PSUM bank = 512 fp32)
NBLK = N // 128    # 32 key blocks
NPAIR = NBLK // 2  # 16 key-block pairs (one exp tile each)
NSTRIP = MQ // MQ_CHUNK  # 4

F32 = mybir.dt.float32
BF16 = mybir.dt.bfloat16
FP16 = mybir.dt.float16
FP8 = mybir.dt.float8e4
I16 = mybir.dt.int16

# Static fp8 quantization scales (powers of two) for the projections.
S_X = 16.0    # x -> fp8
S_W = 64.0    # weights -> fp8
A_Q = 1.0 / (S_W * S_X * 16.0)   # psq -> q/sqrt(C) in bf16
A_K = 1.0 / (S_W * S_X)          # psk -> k in bf16
A_V = 1.0 / (S_W * S_X)          # psv -> gamma*v in bf16

# Schraudolph exp on DVE: y = round(E*GAMMA + BETA); int16 y bitcast to
# bf16 approximates exp(E) with a zero-mean ~0.25% sawtooth.
EXP_GAMMA = 184.6649652337873
EXP_BETA = 16248.665434859407

WARMUP_MMS = int(os.environ.get("KERNEL_WARMUP_MMS", "7"))
# ACT-tiles per strip (of 16 exp pair-tiles); the rest go to DVE.
_split = os.environ.get("KERNEL_EXP_SPLIT", "12,8,8,8")
ACT_TILES = [int(x) for x in _split.split(",")]
# How many of the 16 V-copy pairs run on ACT (rest on DVE).
VCOPY_ACT = int(os.environ.get("KERNEL_VCOPY_ACT", "8"))

# Module-level stash of the last run's results (exec_time_ns etc.) so a
# test harness can report HW time without changing kernel()'s signature.
last_results = None
_nc_cache = {}


def _dve_pairs(st):
    """Which exp pair-tiles of strip st run on DVE (spread evenly)."""
    n_dve = 16 - ACT_TILES[st]
    if n_dve <= 0:
        return set()
    step = 16 / n_dve
    return {min(15, int(step * (i + 0.5))) for i in range(n_dve)}


def _build_nc(has_bq, has_bk, has_bv):
    nc = bacc.Bacc(None)

    # xb is the core's batch with its 2048 query columns rotated to the
    # front (softmax over keys is permutation-invariant), so the query
    # slice is the compile-time-constant columns 0:MQ of xb.
    xb_d = nc.declare_dram_parameter("xb8", [C, N], FP8, isOutput=False)
    xqt_d = nc.declare_dram_parameter("xqt", [MQ, C], FP16, isOutput=False)
    wqt_d = nc.declare_dram_parameter("wqt4", [C, 128], FP8, isOutput=False)
    wkt_d = nc.declare_dram_parameter("wkt4", [C, 128], FP8, isOutput=False)
    wvt_d = nc.declare_dram_parameter("wvt", [C, C], FP8, isOutput=False)
    if has_bq:
        bq_d = nc.declare_dram_parameter("bq4", [128, 1], F32, isOutput=False)
    if has_bk:
        bk_d = nc.declare_dram_parameter("bk4", [128, 1], F32, isOutput=False)
    if has_bv:
        bv_d = nc.declare_dram_parameter("bvg", [128, 512], F32, isOutput=False)
    out_d = nc.declare_dram_parameter("out", [MQ, C], FP16, isOutput=True)

    with TileContext(nc) as tc:
        with (
            tc.tile_pool(name="const", bufs=1) as const,
            tc.tile_pool(name="acts", bufs=1) as acts,
            tc.tile_pool(name="ptp", bufs=18) as ptp,
            tc.tile_pool(name="outp", bufs=3) as outp,
        ):
            # ---- load weights + activations --------------------------------
            wq_sb = const.tile([128, 2, 128], FP8)
            wk_sb = const.tile([128, 2, 128], FP8)
            wv_sb = const.tile([128, 2, C], FP8)
            xb_sb = acts.tile([128, 2, N], FP8)
            nc.sync.dma_start(out=wq_sb, in_=wqt_d[:].rearrange("(t p) m -> p t m", p=128))
            nc.scalar.dma_start(out=wk_sb, in_=wkt_d[:].rearrange("(t p) m -> p t m", p=128))
            for nh in range(8):
                for t in range(2):
                    eng = nc.sync if t == 0 else nc.scalar
                    eng.dma_start(
                        out=xb_sb[:, t, 512 * nh:512 * (nh + 1)],
                        in_=xb_d[t * 128:(t + 1) * 128, 512 * nh:512 * (nh + 1)])
                if nh == 1:
                    nc.scalar.dma_start(
                        out=wv_sb, in_=wvt_d[:].rearrange("(t p) m -> p t m", p=128))
            if has_bq:
                bq_sb = const.tile([128, 1], F32)
                nc.sync.dma_start(out=bq_sb, in_=bq_d[:, :])
            if has_bk:
                bk_sb = const.tile([128, 1], F32)
                nc.sync.dma_start(out=bk_sb, in_=bk_d[:, :])
            if has_bv:
                bv_sb = const.tile([128, 512], F32)
                nc.sync.dma_start(out=bv_sb, in_=bv_d[:, :])
            q_rep = acts.tile([128, MQ], BF16)
            k_rep = acts.tile([128, N], BF16)
            # vhat[p, blk, c]: V^T with an extra ones-column so the P@V
            # PSUM column 256 accumulates the softmax denominator.
            vhat = acts.tile([128, NBLK, C + 1], BF16)
            # residual x^T in fp16, prefetched (needed from strip 0's output)
            xqt_sb = acts.tile([128, 16, C], FP16)
            nc.scalar.dma_start(
                out=xqt_sb,
                in_=xqt_d[:].rearrange("(qb p) m -> p qb m", p=128))

            # 4-slot PSUM arena shared by ALL PSUM producers except the AV
            # accumulators: Q/K/V projection outputs and E^T tiles each take
            # one [128, 512] bank-slot transiently (subtile deps order the
            # producers/consumers); pso gets the other 4 banks.
            psum_e = tc.alloc_tile_pool(name="psum_e", bufs=1, space="PSUM")
            earena = psum_e.tile([128, 4, 512], F32, name="earena")
            psum_o = tc.alloc_tile_pool(name="psum_o", bufs=1, space="PSUM")
            pts = {}  # (st, pair) -> pt tile [128, 1024] bf16
            pso_by_strip = {}

            def emit_e_group(st, g):
                """Energy^T for key blocks 4g..4g+3 of strip st (4-way PE
                row-tiling: all four 32-row matmuls run concurrently)."""
                qsl = slice(MQ_CHUNK * st, MQ_CHUNK * (st + 1))
                for j in range(4):
                    blk = 4 * g + j
                    nc.tensor.matmul(
                        earena[:, j, :],
                        lhsT=k_rep[32 * j:32 * (j + 1), 128 * blk:128 * (blk + 1)],
                        rhs=q_rep[32 * j:32 * (j + 1), qsl],
                        start=True, stop=True,
                        tile_position=(32 * j, 0),
                        skip_group_check=True,
                    )

            def emit_exp(st, pair):
                """exp of pair-tile -> bf16 P^T [128, 1024] (2 blocks x 512q)."""
                j0 = (2 * pair) % 4
                src = earena[:, j0:j0 + 2, :]
                pt = ptp.tile([128, 1024], BF16, tag="pt", name="pt")
                if pair in _dve_pairs(st):
                    nc.vector.tensor_scalar(
                        pt.bitcast(I16), src, EXP_GAMMA, EXP_BETA,
                        mybir.AluOpType.mult, mybir.AluOpType.add)
                else:
                    nc.scalar.activation(pt, src,
                                         func=mybir.ActivationFunctionType.Exp)
                pts[(st, pair)] = pt

            def emit_k(mc, slot):
                """K-projection chunk mc (512 keys) through arena slot."""
                psk = earena[:, slot, :]
                sl = slice(512 * mc, 512 * (mc + 1))
                nc.tensor.matmul(psk, lhsT=wk_sb, rhs=xb_sb[:, :, sl],
                                 start=True, stop=True,
                                 perf_mode=mybir.MatmulPerfMode.DoubleRow,
                                 skip_group_check=True)
                if has_bk:
                    nc.vector.tensor_scalar(k_rep[:, sl], psk, A_K, bk_sb,
                                            mybir.AluOpType.mult,
                                            mybir.AluOpType.add)
                else:
                    nc.vector.tensor_scalar_mul(k_rep[:, sl], psk, A_K)

            def emit_v(pv, slot, on_act):
                """V^T projection pair pv (key blocks 2pv, 2pv+1)."""
                psv = earena[:, slot, :]
                for half in range(2):
                    nb = 2 * pv + half
                    nc.tensor.matmul(
                        psv[:, 256 * half:256 * (half + 1)],
                        lhsT=xb_sb[:, :, 128 * nb:128 * (nb + 1)],
                        rhs=wv_sb,
                        start=True, stop=True,
                        perf_mode=mybir.MatmulPerfMode.DoubleRow,
                        skip_group_check=True)
                dst = vhat[:, 2 * pv:2 * pv + 2, 0:C]
                if has_bv:
                    nc.vector.scalar_tensor_tensor(
                        dst, psv, A_V, bv_sb,
                        op0=mybir.AluOpType.mult, op1=mybir.AluOpType.add)
                elif on_act:
                    nc.scalar.activation(dst, psv,
                                         func=mybir.ActivationFunctionType.Copy,
                                         scale=A_V)
                else:
                    nc.vector.tensor_scalar_mul(dst, psv, A_V)

            def emit_av(st, pair):
                """Accumulate P@[V^T|1] for key blocks 2pair, 2pair+1."""
                if pair == 0:
                    pso_by_strip[st] = [
                        psum_o.tile([128, C + 1], F32, tag=f"o{s}", name=f"pso{s}")
                        for s in range(4)]
                pso = pso_by_strip[st]
                pt = pts[(st, pair)]
                for s in range(4):
                    for half in range(2):
                        blk = 2 * pair + half
                        nc.tensor.matmul(
                            pso[s], lhsT=pt[:, 512 * half + 128 * s:512 * half + 128 * (s + 1)],
                            rhs=vhat[:, blk, :],
                            start=(blk == 0), stop=(blk == NBLK - 1),
                        )
                del pts[(st, pair)]

            def emit_out(st):
                """Normalize + residual + store strip st (fp16)."""
                pso = pso_by_strip.pop(st)
                osb = outp.tile([128, 4, C], FP16, tag="osb", name="osb")
                for s in range(4):
                    qb = 4 * st + s
                    rec = outp.tile([128, 1], F32, tag="rec", name="rec")
                    nc.vector.reciprocal(rec, pso[s][:, C:C + 1])
                    nc.vector.scalar_tensor_tensor(
                        osb[:, s, :], pso[s][:, 0:C], rec, xqt_sb[:, qb, :],
                        op0=mybir.AluOpType.mult, op1=mybir.AluOpType.add)
                nc.sync.dma_start(
                    out=out_d[512 * st:512 * (st + 1), :].rearrange(
                        "(s p) m -> p s m", p=128),
                    in_=osb)

            # ---- prologue: warmup, Q, then K/V/E/exp trickle ----------------
            # PE warm-up while input DMAs are in flight (HAM releases the
            # 2.4 GHz clock after sustained matmul activity) + ACT exp
            # table preload.
            warm = const.tile([128, 512], BF16)
            nc.vector.memset(warm, 0.0)
            warm_exp = const.tile([128, 1], F32)
            nc.scalar.activation(warm_exp, warm[:, 0:1],
                                 func=mybir.ActivationFunctionType.Exp)
            for w in range(WARMUP_MMS):
                nc.tensor.matmul(earena[:, w % 4, :], lhsT=warm[:, 0:128],
                                 rhs=warm, start=True, stop=True,
                                 skip_group_check=True)
            nc.vector.memset(vhat[:, :, C:C + 1], 1.0)
            # Q projection (DoubleRow fp8, contraction 256): psq = S_W*S_X*q
            for mc in range(MQ // 512):
                psq = earena[:, mc, :]
                sl = slice(512 * mc, 512 * (mc + 1))
                nc.tensor.matmul(psq, lhsT=wq_sb, rhs=xb_sb[:, :, sl],
                                 start=True, stop=True,
                                 perf_mode=mybir.MatmulPerfMode.DoubleRow,
                                 skip_group_check=True)
                if has_bq:
                    nc.vector.tensor_scalar(q_rep[:, sl], psq, A_Q, bq_sb,
                                            mybir.AluOpType.mult,
                                            mybir.AluOpType.add)
                else:
                    nc.vector.tensor_scalar_mul(q_rep[:, sl], psq, A_Q)

            # Head iterations it=0..3: K chunks + E group + exp + V pairs,
            # with strip-0 AV trailing one iteration behind the V copies.
            # Then the uniform loop: 2 AV pairs + next E group per slot.
            for it in range(4):
                emit_k(2 * it, 0)
                emit_k(2 * it + 1, 1)
                emit_e_group(0, it)
                emit_exp(0, 2 * it)
                emit_exp(0, 2 * it + 1)
                for i, pv in enumerate(range(4 * it, 4 * it + 4)):
                    emit_v(pv, i, on_act=(pv % 2 == 0) if VCOPY_ACT == 8
                           else (pv < VCOPY_ACT))
                if it >= 2:
                    emit_av(0, 2 * (it - 2))
                    emit_av(0, 2 * (it - 2) + 1)
            # Consumed so far: AV pairs 0..3 (groups 0,1). E emitted: groups
            # 0..3 of strip 0 (global G=0..3).
            for G in range(2, 32):
                st, g = divmod(G, 8)
                emit_av(st, 2 * g)
                emit_av(st, 2 * g + 1)
                GE = G + 2
                if GE < 32:
                    ste, ge = divmod(GE, 8)
                    emit_e_group(ste, ge)
                    emit_exp(ste, 2 * ge)
                    emit_exp(ste, 2 * ge + 1)
                if g == 7:
                    emit_out(st)
            psum_o.release()
            psum_e.release()

    if not nc.is_finalized():
        nc.finalize()
    return nc


def kernel(x, Wq, bq, Wk, bk, Wv, bv, gamma):
    global last_results
    x = np.asarray(x, dtype=np.float32)
    Wq = np.asarray(Wq, dtype=np.float32)
    Wk = np.asarray(Wk, dtype=np.float32)
    Wv = np.asarray(Wv, dtype=np.float32)
    bq = np.asarray(bq, dtype=np.float32)
    bk = np.asarray(bk, dtype=np.float32)
    bv = np.asarray(bv, dtype=np.float32)
    gamma_v = float(np.asarray(gamma).reshape(-1)[0])
    assert x.shape == (B, C, H, W)

    has_bq = bool(np.any(bq != 0))
    has_bk = bool(np.any(bk != 0))
    has_bv = bool(np.any(bv != 0))

    key = (has_bq, has_bk, has_bv)
    if key not in _nc_cache:
        _nc_cache[key] = _build_nc(*key)
    nc = _nc_cache[key]

    f8 = ml_dtypes.float8_e4m3
    wqt4 = np.tile(Wq.T * S_W, (1, 4)).astype(f8)            # [C, 128]
    wkt4 = np.tile(Wk.T * S_W, (1, 4)).astype(f8)            # [C, 128]
    wvt = (Wv.T * (gamma_v * S_W)).astype(f8)                # [C, C]

    xf = x.reshape(B, C, N)
    in_maps = []
    for core in range(NCORES):
        b, half = divmod(core, 2)
        qsl = slice(half * MQ, (half + 1) * MQ)
        # rotate the core's query columns to the front; softmax over keys is
        # permutation-invariant so key order doesn't matter
        xrot = np.roll(xf[b], -half * MQ, axis=1) if half else xf[b]
        m = {
            "xb8": (xrot * S_X).astype(f8),
            "xqt": np.ascontiguousarray(xf[b][:, qsl].T).astype(np.float16),
            "wqt4": wqt4,
            "wkt4": wkt4,
            "wvt": wvt,
        }
        if has_bq:
            m["bq4"] = (np.tile(bq, 4) / 16.0).reshape(128, 1).astype(np.float32)
        if has_bk:
            m["bk4"] = np.tile(bk, 4).reshape(128, 1).astype(np.float32)
        if has_bv:
            m["bvg"] = np.broadcast_to(
                np.tile(bv * gamma_v, 2), (128, 512)).astype(np.float32).copy()
        in_maps.append(m)

    trace = bool(os.environ.get("BASS_TRACE"))
    if trace:
        try:
            import antenv.axon_hooks  # noqa: F401
        except ImportError:
            trace = False
    tmpdir = os.environ.get("BASS_KERNEL_TMPDIR") or None
    res = run_bass_kernel_spmd(nc, in_maps, list(range(NCORES)), trace=trace,
                               tmpdir=tmpdir)
    last_results = res

    out = np.empty((B, C, N), dtype=np.float32)
    for core in range(NCORES):
        b, half = divmod(core, 2)
        out[b, :, half * MQ:(half + 1) * MQ] = res.results[core]["out"].T.astype(np.float32)
    return out.reshape(B, C, H, W)


# revision 16
# speedup vs baseline: 1.1651x; 1.1651x over previous
"""Fused single-head CNN self-attention kernel for Trainium2 (8 NeuronCores).

Computes, per batch b:
    q = (Wq/sqrt(C)) @ x + bq/sqrt(C)   (Cqk=32, N=4096)
    k = Wk @ x + bk
    v = Wv @ x + bv
    E[i, j]  = q[:, i] . k[:, j]        (already scaled by 1/sqrt(C))
    P        = softmax_j(E)
    out[c,i] = gamma * sum_j P[i, j] v[c, j] + x[c, i]

Sharding: B=4 batches x 2 query-halves -> 8 cores, no cross-core comms.
Each core handles one batch's full keys/values and 2048 queries.

Measured TRN2 facts this kernel is built around:
  * The PE does 16384 MACs/cycle regardless of dtype: fp8 gives NO matmul
    throughput (DoubleRow only packs 2x contraction per instruction).
    So E / P@V stay bf16; the MAC floor is the AV chain (~55us/core).
  * Projections run as fp8 DoubleRow matmuls anyway: same speed, but the
    x input DMA halves to 1MB and instruction count drops.
  * ACT exp runs at 1 elem/cycle/lane @1.2GHz (dtype-independent): the
    65536 exp-columns/core would cost ~68us on ACT alone - more than the
    PE strip time - so exp is split between ACT (true Exp) and DVE
    (Schraudolph bit-trick: int16(E*128*log2e + beta) bitcast to bf16,
    zero-mean ~0.25% sawtooth). GPSIMD cannot read PSUM (only other lane).
  * E^T tiles live in a 4-bank PSUM arena (block b -> bank b%4) so E
    matmuls, exp consumers, and AV overlap without pool coupling.
  * Normalize + residual fuse into one DVE scalar_tensor_tensor
    (pso * recip(denom) + x^T), with x^T prefetched in fp16 and the
    output stored fp16 (host upconverts) - no fp32 residual DMA traffic.
"""

import os

import numpy as np
import ml_dtypes

import concourse.bass as bass
import concourse.mybir as mybir
from concourse import bacc
from concourse.tile import TileContext
from concourse.bass_utils import run_bass_kernel_spmd

# Problem shape (hardcoded per contest contract).
B, C, H, W = 4, 256, 64, 64
N = H * W          # 4096 keys per batch
D = 32             # q/k head dim
NCORES = 8
MQ = N // 2        # 2048 queries per core
MQ_CHUNK = 512     # query strip width (PSUM bank = 512 fp32)
NBLK = N // 128    # 32 key blocks
NPAIR = NBLK // 2  # 16 key-block pairs (one exp tile each)
NSTRIP = MQ // MQ_CHUNK  # 4

F32 = mybir.dt.float32
BF16 = mybir.dt.bfloat16
FP16 = mybir.dt.float16
FP8 = mybir.dt.float8e4
I16 = mybir.dt.int16

# Static fp8 quantization scales (powers of two) for the projections.
S_X = 16.0    # x -> fp8
S_W = 64.0    # weights -> fp8
A_Q = 1.0 / (S_W * S_X * 16.0)   # psq -> q/sqrt(C) in bf16
A_K = 1.0 / (S_W * S_X)          # psk -> k in bf16
A_V = 1.0 / (S_W * S_X)          # psv -> gamma*v in bf16

# Schraudolph exp on DVE: y = round(E*GAMMA + BETA); int16 y bitcast to
# bf16 approximates exp(E) with a zero-mean ~0.25% sawtooth.
EXP_GAMMA = 184.6649652337873
EXP_BETA = 16248.665434859407

WARMUP_MMS = int(os.environ.get("KERNEL_WARMUP_MMS", "7"))
# ACT-tiles per strip (of 16 exp pair-tiles); the rest go to DVE.
_split = os.environ.get("KERNEL_EXP_SPLIT", "12,10,10,10")
ACT_TILES = [int(x) for x in _split.split(",")]
# How many of the 16 V-copy pairs run on ACT (rest on DVE).
VCOPY_ACT = int(os.environ.get("KERNEL_VCOPY_ACT", "6"))

# Module-level stash of the last run's results (exec_time_ns etc.) so a
# test harness can report HW time without changing kernel()'s signature.
last_results = None
_nc_cache = {}


def _dve_pairs(st):
    """Which exp pair-tiles of strip st run on DVE (spread evenly).
    Strip 0's first 8 pairs run in the projection head where DVE is
    saturated with PSUM->SBUF copies, so they are pinned to ACT."""
    lo = 8 if st == 0 else 0
    n = 16 - lo
    n_dve = n - max(0, ACT_TILES[st] - lo)
    if n_dve <= 0:
        return set()
    step = n / n_dve
    return {lo + min(n - 1, int(step * (i + 0.5))) for i in range(n_dve)}


def _build_nc(has_bq, has_bk, has_bv):
    nc = bacc.Bacc(None)

    # xb is the core's batch with its 2048 query columns rotated to the
    # front (softmax over keys is permutation-invariant), so the query
    # slice is the compile-time-constant columns 0:MQ of xb.
    xb_d = nc.declare_dram_parameter("xb8", [C, N], FP8, isOutput=False)
    xqt_d = nc.declare_dram_parameter("xqt", [MQ, C], FP16, isOutput=False)
    wqt_d = nc.declare_dram_parameter("wqt4", [C, 128], FP8, isOutput=False)
    wkt_d = nc.declare_dram_parameter("wkt4", [C, 128], FP8, isOutput=False)
    wvt_d = nc.declare_dram_parameter("wvt", [C, C], FP8, isOutput=False)
    if has_bq:
        bq_d = nc.declare_dram_parameter("bq4", [128, 1], F32, isOutput=False)
    if has_bk:
        bk_d = nc.declare_dram_parameter("bk4", [128, 1], F32, isOutput=False)
    if has_bv:
        bv_d = nc.declare_dram_parameter("bvg", [128, 512], F32, isOutput=False)
    out_d = nc.declare_dram_parameter("out", [MQ, C], FP16, isOutput=True)

    with TileContext(nc) as tc:
        with (
            tc.tile_pool(name="const", bufs=1) as const,
            tc.tile_pool(name="acts", bufs=1) as acts,
            tc.tile_pool(name="ptp", bufs=18) as ptp,
            tc.tile_pool(name="outp", bufs=3) as outp,
        ):
            # ---- load weights + activations --------------------------------
            wq_sb = const.tile([128, 2, 128], FP8)
            wk_sb = const.tile([128, 2, 128], FP8)
            wv_sb = const.tile([128, 2, C], FP8)
            xb_sb = acts.tile([128, 2, N], FP8)
            nc.sync.dma_start(out=wq_sb, in_=wqt_d[:].rearrange("(t p) m -> p t m", p=128))
            nc.scalar.dma_start(out=wk_sb, in_=wkt_d[:].rearrange("(t p) m -> p t m", p=128))
            for nh in range(8):
                for t in range(2):
                    eng = nc.sync if t == 0 else nc.scalar
                    eng.dma_start(
                        out=xb_sb[:, t, 512 * nh:512 * (nh + 1)],
                        in_=xb_d[t * 128:(t + 1) * 128, 512 * nh:512 * (nh + 1)])
                if nh == 1:
                    nc.scalar.dma_start(
                        out=wv_sb, in_=wvt_d[:].rearrange("(t p) m -> p t m", p=128))
            if has_bq:
                bq_sb = const.tile([128, 1], F32)
                nc.sync.dma_start(out=bq_sb, in_=bq_d[:, :])
            if has_bk:
                bk_sb = const.tile([128, 1], F32)
                nc.sync.dma_start(out=bk_sb, in_=bk_d[:, :])
            if has_bv:
                bv_sb = const.tile([128, 512], F32)
                nc.sync.dma_start(out=bv_sb, in_=bv_d[:, :])
            q_rep = acts.tile([128, MQ], BF16)
            k_rep = acts.tile([128, N], BF16)
            # vhat[p, blk, c]: V^T with an extra ones-column so the P@V
            # PSUM column 256 accumulates the softmax denominator.
            vhat = acts.tile([128, NBLK, C + 1], BF16)
            # residual x^T in fp16, prefetched (needed from strip 0's output)
            xqt_sb = acts.tile([128, 16, C], FP16)
            nc.scalar.dma_start(
                out=xqt_sb,
                in_=xqt_d[:].rearrange("(qb p) m -> p qb m", p=128))

            # 4-slot PSUM arena for E^T tiles: block b -> slot b%4; an exp
            # pair-instruction consumes two adjacent slots as [128, 1024].
            psum_e = tc.alloc_tile_pool(name="psum_e", bufs=1, space="PSUM")
            earena = psum_e.tile([128, 4, 512], F32, name="earena")
            pts = {}  # (st, pair) -> pt tile [128, 1024] bf16
            pso_by_strip = {}

            def emit_e_group(st, g):
                """Energy^T for key blocks 4g..4g+3 of strip st (4-way PE
                row-tiling: all four 32-row matmuls run concurrently)."""
                qsl = slice(MQ_CHUNK * st, MQ_CHUNK * (st + 1))
                for j in range(4):
                    blk = 4 * g + j
                    nc.tensor.matmul(
                        earena[:, j, :],
                        lhsT=k_rep[32 * j:32 * (j + 1), 128 * blk:128 * (blk + 1)],
                        rhs=q_rep[32 * j:32 * (j + 1), qsl],
                        start=True, stop=True,
                        tile_position=(32 * j, 0),
                        skip_group_check=True,
                    )

            def emit_exp(st, pair):
                """exp of pair-tile -> bf16 P^T [128, 1024] (2 blocks x 512q)."""
                j0 = (2 * pair) % 4
                src = earena[:, j0:j0 + 2, :]
                pt = ptp.tile([128, 1024], BF16, tag="pt", name="pt")
                if pair in _dve_pairs(st):
                    nc.vector.tensor_scalar(
                        pt.bitcast(I16), src, EXP_GAMMA, EXP_BETA,
                        mybir.AluOpType.mult, mybir.AluOpType.add)
                else:
                    nc.scalar.activation(pt, src,
                                         func=mybir.ActivationFunctionType.Exp)
                pts[(st, pair)] = pt

            def emit_av(st, pair):
                """Accumulate P@[V^T|1] for key blocks 2pair, 2pair+1."""
                if pair == 0:
                    pso_by_strip[st] = psum_o.tile(
                        [128, 4, C + 1], F32, tag="pso", name="pso")
                pso = pso_by_strip[st]
                pt = pts[(st, pair)]
                for s in range(4):
                    for half in range(2):
                        blk = 2 * pair + half
                        nc.tensor.matmul(
                            pso[:, s, :],
                            lhsT=pt[:, 512 * half + 128 * s:512 * half + 128 * (s + 1)],
                            rhs=vhat[:, blk, :],
                            start=(blk == 0), stop=(blk == NBLK - 1),
                            skip_group_check=True,
                        )
                del pts[(st, pair)]

            def emit_out(st):
                """Normalize + residual + store strip st (fp16)."""
                pso = pso_by_strip.pop(st)
                osb = outp.tile([128, 4, C], FP16, tag="osb", name="osb")
                for s in range(4):
                    qb = 4 * st + s
                    rec = outp.tile([128, 1], F32, tag="rec", name="rec")
                    nc.vector.reciprocal(rec, pso[:, s, C:C + 1])
                    nc.vector.scalar_tensor_tensor(
                        osb[:, s, :], pso[:, s, 0:C], rec, xqt_sb[:, qb, :],
                        op0=mybir.AluOpType.mult, op1=mybir.AluOpType.add)
                nc.sync.dma_start(
                    out=out_d[512 * st:512 * (st + 1), :].rearrange(
                        "(s p) m -> p s m", p=128),
                    in_=osb)

            with tc.tile_pool(name="psum_p", bufs=2, space="PSUM") as psum_p:
                # PE warm-up while input DMAs are in flight (HAM releases the
                # 2.4 GHz clock after sustained matmul activity) + ACT exp
                # table preload.
                warm = const.tile([128, 512], BF16)
                nc.vector.memset(warm, 0.0)
                warm_exp = const.tile([128, 1], F32)
                nc.scalar.activation(warm_exp, warm[:, 0:1],
                                     func=mybir.ActivationFunctionType.Exp)
                for _ in range(WARMUP_MMS):
                    psw = psum_p.tile([128, 512], F32, tag="pp", name="psw")
                    nc.tensor.matmul(psw, lhsT=warm[:, 0:128], rhs=warm,
                                     start=True, stop=True)
                nc.vector.memset(vhat[:, :, C:C + 1], 1.0)
                # Q projection (DoubleRow fp8, contraction 256)
                for mc in range(MQ // 512):
                    psq = psum_p.tile([128, 512], F32, tag="pp")
                    sl = slice(512 * mc, 512 * (mc + 1))
                    nc.tensor.matmul(psq, lhsT=wq_sb, rhs=xb_sb[:, :, sl],
                                     start=True, stop=True,
                                     perf_mode=mybir.MatmulPerfMode.DoubleRow)
                    if has_bq:
                        nc.vector.tensor_scalar(q_rep[:, sl], psq, A_Q, bq_sb,
                                                mybir.AluOpType.mult,
                                                mybir.AluOpType.add)
                    else:
                        nc.vector.tensor_scalar_mul(q_rep[:, sl], psq, A_Q)
                # K/V projections + strip-0 E/exp. Engine lanes are kept
                # disjoint: all PSUM->SBUF projection copies on DVE, all
                # strip-0 exps on ACT, so the E-chain (k-copy -> E -> exp)
                # only shares the PE with the V pipeline. V matmuls are
                # emitted before each E group so the in-order PE stream
                # never stalls on exp while V work is available.
                for qt in range(4):
                    for mc in range(2 * qt, 2 * qt + 2):
                        psk = psum_p.tile([128, 512], F32, tag="pp")
                        sl = slice(512 * mc, 512 * (mc + 1))
                        nc.tensor.matmul(psk, lhsT=wk_sb, rhs=xb_sb[:, :, sl],
                                         start=True, stop=True,
                                         perf_mode=mybir.MatmulPerfMode.DoubleRow)
                        if has_bk:
                            nc.vector.tensor_scalar(k_rep[:, sl], psk, A_K, bk_sb,
                                                    mybir.AluOpType.mult,
                                                    mybir.AluOpType.add)
                        else:
                            nc.vector.tensor_scalar_mul(k_rep[:, sl], psk, A_K)
                    for pv in range(4 * qt, 4 * qt + 4):
                        # psv covers key blocks 2pv, 2pv+1
                        psv = psum_p.tile([128, 512], F32, tag="pv")
                        for half in range(2):
                            nb = 2 * pv + half
                            nc.tensor.matmul(
                                psv[:, 256 * half:256 * (half + 1)],
                                lhsT=xb_sb[:, :, 128 * nb:128 * (nb + 1)],
                                rhs=wv_sb,
                                start=True, stop=True,
                                perf_mode=mybir.MatmulPerfMode.DoubleRow,
                                skip_group_check=True)
                        dst = vhat[:, 2 * pv:2 * pv + 2, 0:C]
                        if has_bv:
                            nc.vector.scalar_tensor_tensor(
                                dst, psv, A_V, bv_sb,
                                op0=mybir.AluOpType.mult,
                                op1=mybir.AluOpType.add)
                        elif pv < VCOPY_ACT:
                            nc.scalar.activation(
                                dst, psv,
                                func=mybir.ActivationFunctionType.Copy,
                                scale=A_V)
                        else:
                            nc.vector.tensor_scalar_mul(dst, psv, A_V)
                    emit_e_group(0, qt)
                    emit_exp(0, 2 * qt)
                    emit_exp(0, 2 * qt + 1)

            # ---- attention strips (one flat cross-strip pipeline) ----------
            psum_o = tc.alloc_tile_pool(name="psum_o", bufs=1, space="PSUM")
            for G in range(32):
                st, g = divmod(G, 8)
                emit_av(st, 2 * g)
                emit_av(st, 2 * g + 1)
                GE = G + 4
                if GE < 32:
                    ste, ge = divmod(GE, 8)
                    emit_e_group(ste, ge)
                    emit_exp(ste, 2 * ge + 1)
                    emit_exp(ste, 2 * ge)
                if g == 7:
                    emit_out(st)
            psum_o.release()
            psum_e.release()

    if not nc.is_finalized():
        nc.finalize()
    return nc


def kernel(x, Wq, bq, Wk, bk, Wv, bv, gamma):
    global last_results
    x = np.asarray(x, dtype=np.float32)
    Wq = np.asarray(Wq, dtype=np.float32)
    Wk = np.asarray(Wk, dtype=np.float32)
    Wv = np.asarray(Wv, dtype=np.float32)
    bq = np.asarray(bq, dtype=np.float32)
    bk = np.asarray(bk, dtype=np.float32)
    bv = np.asarray(bv, dtype=np.float32)
    gamma_v = float(np.asarray(gamma).reshape(-1)[0])
    assert x.shape == (B, C, H, W)

    has_bq = bool(np.any(bq != 0))
    has_bk = bool(np.any(bk != 0))
    has_bv = bool(np.any(bv != 0))

    key = (has_bq, has_bk, has_bv)
    if key not in _nc_cache:
        _nc_cache[key] = _build_nc(*key)
    nc = _nc_cache[key]

    f8 = ml_dtypes.float8_e4m3
    wqt4 = np.tile(Wq.T * S_W, (1, 4)).astype(f8)            # [C, 128]
    wkt4 = np.tile(Wk.T * S_W, (1, 4)).astype(f8)            # [C, 128]
    wvt = (Wv.T * (gamma_v * S_W)).astype(f8)                # [C, C]

    xf = x.reshape(B, C, N)
    in_maps = []
    for core in range(NCORES):
        b, half = divmod(core, 2)
        qsl = slice(half * MQ, (half + 1) * MQ)
        # rotate the core's query columns to the front; softmax over keys is
        # permutation-invariant so key order doesn't matter
        xrot = np.roll(xf[b], -half * MQ, axis=1) if half else xf[b]
        m = {
            "xb8": (xrot * S_X).astype(f8),
            "xqt": np.ascontiguousarray(xf[b][:, qsl].T).astype(np.float16),
            "wqt4": wqt4,
            "wkt4": wkt4,
            "wvt": wvt,
        }
        if has_bq:
            m["bq4"] = (np.tile(bq, 4) / 16.0).reshape(128, 1).astype(np.float32)
        if has_bk:
            m["bk4"] = np.tile(bk, 4).reshape(128, 1).astype(np.float32)
        if has_bv:
            m["bvg"] = np.broadcast_to(
                np.tile(bv * gamma_v, 2), (128, 512)).astype(np.float32).copy()
        in_maps.append(m)

    trace = bool(os.environ.get("BASS_TRACE"))
    if trace:
        try:
            import antenv.axon_hooks  # noqa: F401
        except ImportError:
            trace = False
    tmpdir = os.environ.get("BASS_KERNEL_TMPDIR") or None
    res = run_bass_kernel_spmd(nc, in_maps, list(range(NCORES)), trace=trace,
                               tmpdir=tmpdir)
    last_results = res

    out = np.empty((B, C, N), dtype=np.float32)
    for core in range(NCORES):
        b, half = divmod(core, 2)
        out[b, :, half * MQ:(half + 1) * MQ] = res.results[core]["out"].T.astype(np.float32)
    return out.reshape(B, C, H, W)


# revision 18
# speedup vs baseline: 1.2042x; 1.0336x over previous
"""Fused single-head CNN self-attention kernel for Trainium2 (8 NeuronCores).

Computes, per batch b:
    q = (Wq/sqrt(C)) @ x + bq/sqrt(C)   (Cqk=32, N=4096)
    k = Wk @ x + bk
    v = Wv @ x + bv
    E[i, j]  = q[:, i] . k[:, j]        (already scaled by 1/sqrt(C))
    P        = softmax_j(E)
    out[c,i] = gamma * sum_j P[i, j] v[c, j] + x[c, i]

Sharding: B=4 batches x 2 query-halves -> 8 cores, no cross-core comms.
Each core handles one batch's full keys/values and 2048 queries.

Measured TRN2 facts this kernel is built around:
  * The PE does 16384 MACs/cycle regardless of dtype: fp8 gives NO matmul
    throughput (DoubleRow only packs 2x contraction per instruction).
    So E / P@V stay bf16; the MAC floor is the AV chain (~55us/core).
  * Projections run as fp8 DoubleRow matmuls anyway: same speed, but the
    x input DMA halves to 1MB and instruction count drops.
  * ACT exp runs at 1 elem/cycle/lane @1.2GHz (dtype-independent): the
    65536 exp-columns/core would cost ~68us on ACT alone - more than the
    PE strip time - so exp is split between ACT (true Exp) and DVE
    (Schraudolph bit-trick: int16(E*128*log2e + beta) bitcast to bf16,
    zero-mean ~0.25% sawtooth). GPSIMD cannot read PSUM (only other lane).
  * E^T tiles live in a 4-bank PSUM arena (block b -> bank b%4) so E
    matmuls, exp consumers, and AV overlap without pool coupling.
  * Normalize + residual fuse into one DVE scalar_tensor_tensor
    (pso * recip(denom) + x^T), with x^T prefetched in fp16 and the
    output stored fp16 (host upconverts) - no fp32 residual DMA traffic.
"""

import os

import numpy as np
import ml_dtypes

import concourse.bass as bass
import concourse.mybir as mybir
from concourse import bacc
from concourse.tile import TileContext
from concourse.bass_utils import run_bass_kernel_spmd

# Problem shape (hardcoded per contest contract).
B, C, H, W = 4, 256, 64, 64
N = H * W          # 4096 keys per batch
D = 32             # q/k head dim
NCORES = 8
MQ = N // 2        # 2048 queries per core
MQ_CHUNK = 512     # query strip width (PSUM bank = 512 fp32)
NBLK = N // 128    # 32 key blocks
NPAIR = NBLK // 2  # 16 key-block pairs (one exp tile each)
NSTRIP = MQ // MQ_CHUNK  # 4

F32 = mybir.dt.float32
BF16 = mybir.dt.bfloat16
FP16 = mybir.dt.float16
FP8 = mybir.dt.float8e4
I16 = mybir.dt.int16

# Static fp8 quantization scales (powers of two) for the projections.
S_X = 16.0    # x -> fp8
S_W = 64.0    # weights -> fp8
A_Q = 1.0 / (S_W * S_X * 16.0)   # psq -> q/sqrt(C) in bf16
A_K = 1.0 / (S_W * S_X)          # psk -> k in bf16
A_V = 1.0 / (S_W * S_X)          # psv -> gamma*v in bf16

# Schraudolph exp on DVE: y = round(E*GAMMA + BETA); int16 y bitcast to
# bf16 approximates exp(E) with a zero-mean ~0.25% sawtooth.
EXP_GAMMA = 184.6649652337873
EXP_BETA = 16248.665434859407

WARMUP_MMS = int(os.environ.get("KERNEL_WARMUP_MMS", "7"))
# ACT-tiles per strip (of 16 exp pair-tiles); the rest go to DVE.
_split = os.environ.get("KERNEL_EXP_SPLIT", "12,10,10,10")
ACT_TILES = [int(x) for x in _split.split(",")]
# How many of the 16 V-copy pairs run on ACT (rest on DVE).
VCOPY_ACT = int(os.environ.get("KERNEL_VCOPY_ACT", "6"))

# Module-level stash of the last run's results (exec_time_ns etc.) so a
# test harness can report HW time without changing kernel()'s signature.
last_results = None
_nc_cache = {}


def _dve_pairs(st):
    """Which exp pair-tiles of strip st run on DVE (spread evenly).
    Strip 0's first 8 pairs run in the projection head where DVE is
    saturated with PSUM->SBUF copies, so they are pinned to ACT."""
    lo = 8 if st == 0 else 0
    n = 16 - lo
    n_dve = n - max(0, ACT_TILES[st] - lo)
    if n_dve <= 0:
        return set()
    step = n / n_dve
    return {lo + min(n - 1, int(step * (i + 0.5))) for i in range(n_dve)}


def _build_nc(has_bq, has_bk, has_bv):
    nc = bacc.Bacc(None)

    # xb is the core's batch with its 2048 query columns rotated to the
    # front (softmax over keys is permutation-invariant), so the query
    # slice is the compile-time-constant columns 0:MQ of xb.
    xb_d = nc.declare_dram_parameter("xb8", [C, N], FP8, isOutput=False)
    xqt_d = nc.declare_dram_parameter("xqt", [MQ, C], FP16, isOutput=False)
    wqt_d = nc.declare_dram_parameter("wqt4", [C, 128], FP8, isOutput=False)
    wkt_d = nc.declare_dram_parameter("wkt4", [C, 128], FP8, isOutput=False)
    wvt_d = nc.declare_dram_parameter("wvt", [C, C], FP8, isOutput=False)
    if has_bq:
        bq_d = nc.declare_dram_parameter("bq4", [128, 1], F32, isOutput=False)
    if has_bk:
        bk_d = nc.declare_dram_parameter("bk4", [128, 1], F32, isOutput=False)
    if has_bv:
        bv_d = nc.declare_dram_parameter("bvg", [128, 512], F32, isOutput=False)
    out_d = nc.declare_dram_parameter("out", [MQ, C], FP16, isOutput=True)

    with TileContext(nc) as tc:
        with (
            tc.tile_pool(name="const", bufs=1) as const,
            tc.tile_pool(name="acts", bufs=1) as acts,
            tc.tile_pool(name="ptp", bufs=18) as ptp,
            tc.tile_pool(name="outp", bufs=3) as outp,
        ):
            # ---- load weights + activations --------------------------------
            wq_sb = const.tile([128, 2, 128], FP8)
            wk_sb = const.tile([128, 2, 128], FP8)
            wv_sb = const.tile([128, 2, C], FP8)
            xb_sb = acts.tile([128, 2, N], FP8)
            nc.sync.dma_start(out=wq_sb, in_=wqt_d[:].rearrange("(t p) m -> p t m", p=128))
            nc.scalar.dma_start(out=wk_sb, in_=wkt_d[:].rearrange("(t p) m -> p t m", p=128))
            for nh in range(8):
                for t in range(2):
                    eng = nc.sync if t == 0 else nc.scalar
                    eng.dma_start(
                        out=xb_sb[:, t, 512 * nh:512 * (nh + 1)],
                        in_=xb_d[t * 128:(t + 1) * 128, 512 * nh:512 * (nh + 1)])
                if nh == 1:
                    nc.scalar.dma_start(
                        out=wv_sb, in_=wvt_d[:].rearrange("(t p) m -> p t m", p=128))
            if has_bq:
                bq_sb = const.tile([128, 1], F32)
                nc.sync.dma_start(out=bq_sb, in_=bq_d[:, :])
            if has_bk:
                bk_sb = const.tile([128, 1], F32)
                nc.sync.dma_start(out=bk_sb, in_=bk_d[:, :])
            if has_bv:
                bv_sb = const.tile([128, 512], F32)
                nc.sync.dma_start(out=bv_sb, in_=bv_d[:, :])
            q_rep = acts.tile([128, MQ], BF16)
            k_rep = acts.tile([128, N], BF16)
            # vhat[p, blk, c]: V^T with an extra ones-column so the P@V
            # PSUM column 256 accumulates the softmax denominator.
            vhat = acts.tile([128, NBLK, C + 1], BF16)
            # residual x^T in fp16, prefetched (needed from strip 0's output)
            xqt_sb = acts.tile([128, 16, C], FP16)
            nc.scalar.dma_start(
                out=xqt_sb,
                in_=xqt_d[:].rearrange("(qb p) m -> p qb m", p=128))

            # 4-slot PSUM arena for E^T tiles: block b -> slot b%4; an exp
            # pair-instruction consumes two adjacent slots as [128, 1024].
            psum_e = tc.alloc_tile_pool(name="psum_e", bufs=1, space="PSUM")
            earena = psum_e.tile([128, 4, 512], F32, name="earena")
            pts = {}  # (st, pair) -> pt tile [128, 1024] bf16
            pso_by_strip = {}

            def emit_e_group(st, g):
                """Energy^T for key blocks 4g..4g+3 of strip st (4-way PE
                row-tiling: all four 32-row matmuls run concurrently)."""
                qsl = slice(MQ_CHUNK * st, MQ_CHUNK * (st + 1))
                for j in range(4):
                    blk = 4 * g + j
                    nc.tensor.matmul(
                        earena[:, j, :],
                        lhsT=k_rep[32 * j:32 * (j + 1), 128 * blk:128 * (blk + 1)],
                        rhs=q_rep[32 * j:32 * (j + 1), qsl],
                        start=True, stop=True,
                        tile_position=(32 * j, 0),
                        skip_group_check=True,
                    )

            def emit_exp(st, pair):
                """exp of pair-tile -> bf16 P^T [128, 1024] (2 blocks x 512q)."""
                j0 = (2 * pair) % 4
                src = earena[:, j0:j0 + 2, :]
                pt = ptp.tile([128, 1024], BF16, tag="pt", name="pt")
                if pair in _dve_pairs(st):
                    nc.vector.tensor_scalar(
                        pt.bitcast(I16), src, EXP_GAMMA, EXP_BETA,
                        mybir.AluOpType.mult, mybir.AluOpType.add)
                else:
                    nc.scalar.activation(pt, src,
                                         func=mybir.ActivationFunctionType.Exp)
                pts[(st, pair)] = pt

            def emit_av(st, pair):
                """Accumulate P@[V^T|1] for key blocks 2pair, 2pair+1."""
                if pair == 0:
                    pso_by_strip[st] = [
                        psum_o.tile([128, C + 1], F32, tag=f"o{s}", name=f"pso{s}")
                        for s in range(4)]
                pso = pso_by_strip[st]
                pt = pts[(st, pair)]
                for s in range(4):
                    for half in range(2):
                        blk = 2 * pair + half
                        nc.tensor.matmul(
                            pso[s],
                            lhsT=pt[:, 512 * half + 128 * s:512 * half + 128 * (s + 1)],
                            rhs=vhat[:, blk, :],
                            start=(blk == 0), stop=(blk == NBLK - 1),
                        )
                del pts[(st, pair)]

            def emit_out(st):
                """Normalize + residual + store strip st (fp16)."""
                pso = pso_by_strip.pop(st)
                osb = outp.tile([128, 4, C], FP16, tag="osb", name="osb")
                for s in range(4):
                    qb = 4 * st + s
                    rec = outp.tile([128, 1], F32, tag="rec", name="rec")
                    nc.vector.reciprocal(rec, pso[s][:, C:C + 1])
                    nc.vector.scalar_tensor_tensor(
                        osb[:, s, :], pso[s][:, 0:C], rec, xqt_sb[:, qb, :],
                        op0=mybir.AluOpType.mult, op1=mybir.AluOpType.add)
                nc.sync.dma_start(
                    out=out_d[512 * st:512 * (st + 1), :].rearrange(
                        "(s p) m -> p s m", p=128),
                    in_=osb)

            with tc.tile_pool(name="psum_p", bufs=2, space="PSUM") as psum_p:
                # PE warm-up while input DMAs are in flight (HAM releases the
                # 2.4 GHz clock after sustained matmul activity) + ACT exp
                # table preload.
                warm = const.tile([128, 512], BF16)
                nc.vector.memset(warm, 0.0)
                warm_exp = const.tile([128, 1], F32)
                nc.scalar.activation(warm_exp, warm[:, 0:1],
                                     func=mybir.ActivationFunctionType.Exp)
                for _ in range(WARMUP_MMS):
                    psw = psum_p.tile([128, 512], F32, tag="pp", name="psw")
                    nc.tensor.matmul(psw, lhsT=warm[:, 0:128], rhs=warm,
                                     start=True, stop=True)
                nc.vector.memset(vhat[:, :, C:C + 1], 1.0)
                # Q projection (DoubleRow fp8, contraction 256)
                for mc in range(MQ // 512):
                    psq = psum_p.tile([128, 512], F32, tag="pp")
                    sl = slice(512 * mc, 512 * (mc + 1))
                    nc.tensor.matmul(psq, lhsT=wq_sb, rhs=xb_sb[:, :, sl],
                                     start=True, stop=True,
                                     perf_mode=mybir.MatmulPerfMode.DoubleRow)
                    if has_bq:
                        nc.vector.tensor_scalar(q_rep[:, sl], psq, A_Q, bq_sb,
                                                mybir.AluOpType.mult,
                                                mybir.AluOpType.add)
                    else:
                        nc.vector.tensor_scalar_mul(q_rep[:, sl], psq, A_Q)
                # K/V projections + strip-0 E/exp. Engine lanes are kept
                # disjoint: all PSUM->SBUF projection copies on DVE, all
                # strip-0 exps on ACT, so the E-chain (k-copy -> E -> exp)
                # only shares the PE with the V pipeline. V matmuls are
                # emitted before each E group so the in-order PE stream
                # never stalls on exp while V work is available.
                for qt in range(4):
                    for mc in range(2 * qt, 2 * qt + 2):
                        psk = psum_p.tile([128, 512], F32, tag="pp")
                        sl = slice(512 * mc, 512 * (mc + 1))
                        nc.tensor.matmul(psk, lhsT=wk_sb, rhs=xb_sb[:, :, sl],
                                         start=True, stop=True,
                                         perf_mode=mybir.MatmulPerfMode.DoubleRow)
                        if has_bk:
                            nc.vector.tensor_scalar(k_rep[:, sl], psk, A_K, bk_sb,
                                                    mybir.AluOpType.mult,
                                                    mybir.AluOpType.add)
                        else:
                            nc.vector.tensor_scalar_mul(k_rep[:, sl], psk, A_K)
                    for pv in range(4 * qt, 4 * qt + 4):
                        # psv covers key blocks 2pv, 2pv+1
                        psv = psum_p.tile([128, 512], F32, tag="pv")
                        for half in range(2):
                            nb = 2 * pv + half
                            nc.tensor.matmul(
                                psv[:, 256 * half:256 * (half + 1)],
                                lhsT=xb_sb[:, :, 128 * nb:128 * (nb + 1)],
                                rhs=wv_sb,
                                start=True, stop=True,
                                perf_mode=mybir.MatmulPerfMode.DoubleRow,
                                skip_group_check=True)
                        dst = vhat[:, 2 * pv:2 * pv + 2, 0:C]
                        if has_bv:
                            nc.vector.scalar_tensor_tensor(
                                dst, psv, A_V, bv_sb,
                                op0=mybir.AluOpType.mult,
                                op1=mybir.AluOpType.add)
                        elif pv < VCOPY_ACT:
                            nc.scalar.activation(
                                dst, psv,
                                func=mybir.ActivationFunctionType.Copy,
                                scale=A_V)
                        else:
                            nc.vector.tensor_scalar_mul(dst, psv, A_V)
                    emit_e_group(0, qt)
                    emit_exp(0, 2 * qt)
                    emit_exp(0, 2 * qt + 1)

            # ---- attention strips (one flat cross-strip pipeline) ----------
            psum_o = tc.alloc_tile_pool(name="psum_o", bufs=1, space="PSUM")
            for G in range(32):
                st, g = divmod(G, 8)
                emit_av(st, 2 * g)
                emit_av(st, 2 * g + 1)
                GE = G + 4
                if GE < 32:
                    ste, ge = divmod(GE, 8)
                    emit_e_group(ste, ge)
                    emit_exp(ste, 2 * ge + 1)
                    emit_exp(ste, 2 * ge)
                if g == 7:
                    emit_out(st)
            psum_o.release()
            psum_e.release()

    if not nc.is_finalized():
        nc.finalize()
    return nc


def kernel(x, Wq, bq, Wk, bk, Wv, bv, gamma):
    global last_results
    x = np.asarray(x, dtype=np.float32)
    Wq = np.asarray(Wq, dtype=np.float32)
    Wk = np.asarray(Wk, dtype=np.float32)
    Wv = np.asarray(Wv, dtype=np.float32)
    bq = np.asarray(bq, dtype=np.float32)
    bk = np.asarray(bk, dtype=np.float32)
    bv = np.asarray(bv, dtype=np.float32)
    gamma_v = float(np.asarray(gamma).reshape(-1)[0])
    assert x.shape == (B, C, H, W)

    has_bq = bool(np.any(bq != 0))
    has_bk = bool(np.any(bk != 0))
    has_bv = bool(np.any(bv != 0))

    key = (has_bq, has_bk, has_bv)
    if key not in _nc_cache:
        _nc_cache[key] = _build_nc(*key)
    nc = _nc_cache[key]

    f8 = ml_dtypes.float8_e4m3
    wqt4 = np.tile(Wq.T * S_W, (1, 4)).astype(f8)            # [C, 128]
    wkt4 = np.tile(Wk.T * S_W, (1, 4)).astype(f8)            # [C, 128]
    wvt = (Wv.T * (gamma_v * S_W)).astype(f8)                # [C, C]

    xf = x.reshape(B, C, N)
    in_maps = []
    for core in range(NCORES):
        b, half = divmod(core, 2)
        qsl = slice(half * MQ, (half + 1) * MQ)
        # rotate the core's query columns to the front; softmax over keys is
        # permutation-invariant so key order doesn't matter
        xrot = np.roll(xf[b], -half * MQ, axis=1) if half else xf[b]
        m = {
            "xb8": (xrot * S_X).astype(f8),
            "xqt": np.ascontiguousarray(xf[b][:, qsl].T).astype(np.float16),
            "wqt4": wqt4,
            "wkt4": wkt4,
            "wvt": wvt,
        }
        if has_bq:
            m["bq4"] = (np.tile(bq, 4) / 16.0).reshape(128, 1).astype(np.float32)
        if has_bk:
            m["bk4"] = np.tile(bk, 4).reshape(128, 1).astype(np.float32)
        if has_bv:
            m["bvg"] = np.broadcast_to(
                np.tile(bv * gamma_v, 2), (128, 512)).astype(np.float32).copy()
        in_maps.append(m)

    trace = bool(os.environ.get("BASS_TRACE"))
    if trace:
        try:
            import antenv.axon_hooks  # noqa: F401
        except ImportError:
            trace = False
    tmpdir = os.environ.get("BASS_KERNEL_TMPDIR") or None
    res = run_bass_kernel_spmd(nc, in_maps, list(range(NCORES)), trace=trace,
                               tmpdir=tmpdir)
    last_results = res

    out = np.empty((B, C, N), dtype=np.float32)
    for core in range(NCORES):
        b, half = divmod(core, 2)
        out[b, :, half * MQ:(half + 1) * MQ] = res.results[core]["out"].T.astype(np.float32)
    return out.reshape(B, C, H, W)


# revision 21
# speedup vs baseline: 1.2157x; 1.0095x over previous
"""Fused single-head CNN self-attention kernel for Trainium2 (8 NeuronCores).

Computes, per batch b:
    q = (Wq/sqrt(C)) @ x + bq/sqrt(C)   (Cqk=32, N=4096)
    k = Wk @ x + bk
    v = Wv @ x + bv
    E[i, j]  = q[:, i] . k[:, j]        (already scaled by 1/sqrt(C))
    P        = softmax_j(E)
    out[c,i] = gamma * sum_j P[i, j] v[c, j] + x[c, i]

Sharding: B=4 batches x 2 query-halves -> 8 cores, no cross-core comms.
Each core handles one batch's full keys/values and 2048 queries.

Measured TRN2 facts this kernel is built around:
  * The PE does 16384 MACs/cycle regardless of dtype: fp8 gives NO matmul
    throughput (DoubleRow only packs 2x contraction per instruction).
    So E / P@V stay bf16; the MAC floor is the AV chain (~55us/core).
  * Projections run as fp8 DoubleRow matmuls anyway: same speed, but the
    x input DMA halves to 1MB and instruction count drops.
  * ACT exp runs at 1 elem/cycle/lane @1.2GHz (dtype-independent): the
    65536 exp-columns/core would cost ~68us on ACT alone - more than the
    PE strip time - so exp is split between ACT (true Exp) and DVE
    (Schraudolph bit-trick: int16(E*128*log2e + beta) bitcast to bf16,
    zero-mean ~0.25% sawtooth). GPSIMD cannot read PSUM (only other lane).
  * E^T tiles live in a 4-bank PSUM arena (block b -> bank b%4) so E
    matmuls, exp consumers, and AV overlap without pool coupling.
  * Normalize + residual fuse into one DVE scalar_tensor_tensor
    (pso * recip(denom) + x^T), with x^T prefetched in fp16 and the
    output stored fp16 (host upconverts) - no fp32 residual DMA traffic.
"""

import os

import numpy as np
import ml_dtypes

import concourse.bass as bass
import concourse.mybir as mybir
from concourse import bacc
from concourse.tile import TileContext
from concourse.bass_utils import run_bass_kernel_spmd

# Problem shape (hardcoded per contest contract).
B, C, H, W = 4, 256, 64, 64
N = H * W          # 4096 keys per batch
D = 32             # q/k head dim
NCORES = 8
MQ = N // 2        # 2048 queries per core
MQ_CHUNK = 512     # query strip width (PSUM bank = 512 fp32)
NBLK = N // 128    # 32 key blocks
NPAIR = NBLK // 2  # 16 key-block pairs (one exp tile each)
NSTRIP = MQ // MQ_CHUNK  # 4

F32 = mybir.dt.float32
BF16 = mybir.dt.bfloat16
FP16 = mybir.dt.float16
FP8 = mybir.dt.float8e4
I16 = mybir.dt.int16

# Static fp8 quantization scales (powers of two) for the projections.
S_X = 16.0    # x -> fp8
S_W = 64.0    # weights -> fp8
A_Q = 1.0 / (S_W * S_X * 16.0)   # psq -> q/sqrt(C) in bf16
A_K = 1.0 / (S_W * S_X)          # psk -> k in bf16
A_V = 1.0 / (S_W * S_X)          # psv -> gamma*v in bf16

# Schraudolph exp on DVE: y = round(E*GAMMA + BETA); int16 y bitcast to
# bf16 approximates exp(E) with a zero-mean ~0.25% sawtooth.
EXP_GAMMA = 184.6649652337873
EXP_BETA = 16248.665434859407

WARMUP_MMS = int(os.environ.get("KERNEL_WARMUP_MMS", "7"))
# ACT-tiles per strip (of 16 exp pair-tiles); the rest go to DVE.
_split = os.environ.get("KERNEL_EXP_SPLIT", "12,10,10,10")
ACT_TILES = [int(x) for x in _split.split(",")]
# How many of the 16 V-copy pairs run on ACT (rest on DVE).
VCOPY_ACT = int(os.environ.get("KERNEL_VCOPY_ACT", "6"))

# Module-level stash of the last run's results (exec_time_ns etc.) so a
# test harness can report HW time without changing kernel()'s signature.
last_results = None
_nc_cache = {}


def _dve_pairs(st):
    """Which exp pair-tiles of strip st run on DVE (spread evenly).
    Only ODD pairs are eligible: odd pairs use arena slots 2,3, so the
    next E group's first matmuls (slots 0,1) never wait on the slower
    DVE lane. Strip 0's first 8 pairs run in the projection head where
    DVE is saturated with PSUM->SBUF copies, so they stay on ACT."""
    lo = 8 if st == 0 else 0
    odd = [p for p in range(lo, 16) if p % 2 == 1]
    n_dve = min(len(odd), 16 - lo - max(0, ACT_TILES[st] - lo))
    if n_dve <= 0:
        return set()
    step = len(odd) / n_dve
    return {odd[min(len(odd) - 1, int(step * (i + 0.5)))] for i in range(n_dve)}


def _build_nc(has_bq, has_bk, has_bv):
    nc = bacc.Bacc(None)

    # xb is the core's batch with its 2048 query columns rotated to the
    # front (softmax over keys is permutation-invariant), so the query
    # slice is the compile-time-constant columns 0:MQ of xb.
    xb_d = nc.declare_dram_parameter("xb8", [C, N], FP8, isOutput=False)
    xqt_d = nc.declare_dram_parameter("xqt", [MQ, C], FP16, isOutput=False)
    wqt_d = nc.declare_dram_parameter("wqt4", [C, 128], FP8, isOutput=False)
    wkt_d = nc.declare_dram_parameter("wkt4", [C, 128], FP8, isOutput=False)
    wvt_d = nc.declare_dram_parameter("wvt", [C, C], FP8, isOutput=False)
    if has_bq:
        bq_d = nc.declare_dram_parameter("bq4", [128, 1], F32, isOutput=False)
    if has_bk:
        bk_d = nc.declare_dram_parameter("bk4", [128, 1], F32, isOutput=False)
    if has_bv:
        bv_d = nc.declare_dram_parameter("bvg", [128, 512], F32, isOutput=False)
    out_d = nc.declare_dram_parameter("out", [MQ, C], FP16, isOutput=True)

    with TileContext(nc) as tc:
        with (
            tc.tile_pool(name="const", bufs=1) as const,
            tc.tile_pool(name="acts", bufs=1) as acts,
            tc.tile_pool(name="ptp", bufs=18) as ptp,
            tc.tile_pool(name="outp", bufs=3) as outp,
        ):
            # ---- load weights + activations --------------------------------
            wq_sb = const.tile([128, 2, 128], FP8)
            wk_sb = const.tile([128, 2, 128], FP8)
            wv_sb = const.tile([128, 2, C], FP8)
            xb_sb = acts.tile([128, 2, N], FP8)
            nc.sync.dma_start(out=wq_sb, in_=wqt_d[:].rearrange("(t p) m -> p t m", p=128))
            nc.scalar.dma_start(out=wk_sb, in_=wkt_d[:].rearrange("(t p) m -> p t m", p=128))
            for nh in range(8):
                for t in range(2):
                    eng = nc.sync if t == 0 else nc.scalar
                    eng.dma_start(
                        out=xb_sb[:, t, 512 * nh:512 * (nh + 1)],
                        in_=xb_d[t * 128:(t + 1) * 128, 512 * nh:512 * (nh + 1)])
                if nh == 1:
                    nc.scalar.dma_start(
                        out=wv_sb, in_=wvt_d[:].rearrange("(t p) m -> p t m", p=128))
            if has_bq:
                bq_sb = const.tile([128, 1], F32)
                nc.sync.dma_start(out=bq_sb, in_=bq_d[:, :])
            if has_bk:
                bk_sb = const.tile([128, 1], F32)
                nc.sync.dma_start(out=bk_sb, in_=bk_d[:, :])
            if has_bv:
                bv_sb = const.tile([128, 512], F32)
                nc.sync.dma_start(out=bv_sb, in_=bv_d[:, :])
            q_rep = acts.tile([128, MQ], BF16)
            k_rep = acts.tile([128, N], BF16)
            # vhat[p, blk, c]: V^T with an extra ones-column so the P@V
            # PSUM column 256 accumulates the softmax denominator.
            vhat = acts.tile([128, NBLK, C + 1], BF16)
            # residual x^T in fp16, prefetched on the otherwise-idle GPSIMD
            # SWDGE queue so it doesn't delay the xb input DMAs (needed only
            # from strip 0's output, ~30us in)
            xqt_sb = acts.tile([128, 16, C], FP16)
            nc.gpsimd.dma_start(
                out=xqt_sb,
                in_=xqt_d[:].rearrange("(qb p) m -> p qb m", p=128))

            # 4-slot PSUM arena for E^T tiles: block b -> slot b%4; an exp
            # pair-instruction consumes two adjacent slots as [128, 1024].
            psum_e = tc.alloc_tile_pool(name="psum_e", bufs=1, space="PSUM")
            earena = psum_e.tile([128, 4, 512], F32, name="earena")
            pts = {}  # (st, pair) -> pt tile [128, 1024] bf16
            pso_by_strip = {}

            def emit_e_group(st, g):
                """Energy^T for key blocks 4g..4g+3 of strip st (4-way PE
                row-tiling: all four 32-row matmuls run concurrently)."""
                qsl = slice(MQ_CHUNK * st, MQ_CHUNK * (st + 1))
                for j in range(4):
                    blk = 4 * g + j
                    nc.tensor.matmul(
                        earena[:, j, :],
                        lhsT=k_rep[32 * j:32 * (j + 1), 128 * blk:128 * (blk + 1)],
                        rhs=q_rep[32 * j:32 * (j + 1), qsl],
                        start=True, stop=True,
                        tile_position=(32 * j, 0),
                        skip_group_check=True,
                    )

            def emit_exp(st, pair):
                """exp of pair-tile -> bf16 P^T [128, 1024] (2 blocks x 512q)."""
                j0 = (2 * pair) % 4
                src = earena[:, j0:j0 + 2, :]
                pt = ptp.tile([128, 1024], BF16, tag="pt", name="pt")
                if pair in _dve_pairs(st):
                    nc.vector.tensor_scalar(
                        pt.bitcast(I16), src, EXP_GAMMA, EXP_BETA,
                        mybir.AluOpType.mult, mybir.AluOpType.add)
                else:
                    nc.scalar.activation(pt, src,
                                         func=mybir.ActivationFunctionType.Exp)
                pts[(st, pair)] = pt

            def emit_av(st, pair):
                """Accumulate P@[V^T|1] for key blocks 2pair, 2pair+1."""
                if pair == 0:
                    pso_by_strip[st] = [
                        psum_o.tile([128, C + 1], F32, tag=f"o{s}", name=f"pso{s}")
                        for s in range(4)]
                pso = pso_by_strip[st]
                pt = pts[(st, pair)]
                for s in range(4):
                    for half in range(2):
                        blk = 2 * pair + half
                        nc.tensor.matmul(
                            pso[s],
                            lhsT=pt[:, 512 * half + 128 * s:512 * half + 128 * (s + 1)],
                            rhs=vhat[:, blk, :],
                            start=(blk == 0), stop=(blk == NBLK - 1),
                        )
                del pts[(st, pair)]

            def emit_out(st):
                """Normalize + residual + store strip st (fp16)."""
                pso = pso_by_strip.pop(st)
                osb = outp.tile([128, 4, C], FP16, tag="osb", name="osb")
                for s in range(4):
                    qb = 4 * st + s
                    rec = outp.tile([128, 1], F32, tag="rec", name="rec")
                    nc.vector.reciprocal(rec, pso[s][:, C:C + 1])
                    nc.vector.scalar_tensor_tensor(
                        osb[:, s, :], pso[s][:, 0:C], rec, xqt_sb[:, qb, :],
                        op0=mybir.AluOpType.mult, op1=mybir.AluOpType.add)
                nc.sync.dma_start(
                    out=out_d[512 * st:512 * (st + 1), :].rearrange(
                        "(s p) m -> p s m", p=128),
                    in_=osb)

            with tc.tile_pool(name="psum_p", bufs=2, space="PSUM") as psum_p:
                # PE warm-up while input DMAs are in flight (HAM releases the
                # 2.4 GHz clock after sustained matmul activity) + ACT exp
                # table preload.
                warm = const.tile([128, 512], BF16)
                nc.vector.memset(warm, 0.0)
                warm_exp = const.tile([128, 1], F32)
                nc.scalar.activation(warm_exp, warm[:, 0:1],
                                     func=mybir.ActivationFunctionType.Exp)
                for _ in range(WARMUP_MMS):
                    psw = psum_p.tile([128, 512], F32, tag="pp", name="psw")
                    nc.tensor.matmul(psw, lhsT=warm[:, 0:128], rhs=warm,
                                     start=True, stop=True)
                nc.vector.memset(vhat[:, :, C:C + 1], 1.0)
                # Q projection (DoubleRow fp8, contraction 256)
                for mc in range(MQ // 512):
                    psq = psum_p.tile([128, 512], F32, tag="pp")
                    sl = slice(512 * mc, 512 * (mc + 1))
                    nc.tensor.matmul(psq, lhsT=wq_sb, rhs=xb_sb[:, :, sl],
                                     start=True, stop=True,
                                     perf_mode=mybir.MatmulPerfMode.DoubleRow)
                    if has_bq:
                        nc.vector.tensor_scalar(q_rep[:, sl], psq, A_Q, bq_sb,
                                                mybir.AluOpType.mult,
                                                mybir.AluOpType.add)
                    else:
                        nc.vector.tensor_scalar_mul(q_rep[:, sl], psq, A_Q)
                # K/V projections + strip-0 E/exp. Engine lanes are kept
                # disjoint: all PSUM->SBUF projection copies on DVE, all
                # strip-0 exps on ACT, so the E-chain (k-copy -> E -> exp)
                # only shares the PE with the V pipeline. V matmuls are
                # emitted before each E group so the in-order PE stream
                # never stalls on exp while V work is available.
                for qt in range(4):
                    for mc in range(2 * qt, 2 * qt + 2):
                        psk = psum_p.tile([128, 512], F32, tag="pp")
                        sl = slice(512 * mc, 512 * (mc + 1))
                        nc.tensor.matmul(psk, lhsT=wk_sb, rhs=xb_sb[:, :, sl],
                                         start=True, stop=True,
                                         perf_mode=mybir.MatmulPerfMode.DoubleRow)
                        if has_bk:
                            nc.vector.tensor_scalar(k_rep[:, sl], psk, A_K, bk_sb,
                                                    mybir.AluOpType.mult,
                                                    mybir.AluOpType.add)
                        else:
                            nc.vector.tensor_scalar_mul(k_rep[:, sl], psk, A_K)
                    for pv in range(4 * qt, 4 * qt + 4):
                        # psv covers key blocks 2pv, 2pv+1
                        psv = psum_p.tile([128, 512], F32, tag="pv")
                        for half in range(2):
                            nb = 2 * pv + half
                            nc.tensor.matmul(
                                psv[:, 256 * half:256 * (half + 1)],
                                lhsT=xb_sb[:, :, 128 * nb:128 * (nb + 1)],
                                rhs=wv_sb,
                                start=True, stop=True,
                                perf_mode=mybir.MatmulPerfMode.DoubleRow,
                                skip_group_check=True)
                        dst = vhat[:, 2 * pv:2 * pv + 2, 0:C]
                        if has_bv:
                            nc.vector.scalar_tensor_tensor(
                                dst, psv, A_V, bv_sb,
                                op0=mybir.AluOpType.mult,
                                op1=mybir.AluOpType.add)
                        elif pv < VCOPY_ACT:
                            nc.scalar.activation(
                                dst, psv,
                                func=mybir.ActivationFunctionType.Copy,
                                scale=A_V)
                        else:
                            nc.vector.tensor_scalar_mul(dst, psv, A_V)
                    emit_e_group(0, qt)
                    emit_exp(0, 2 * qt)
                    emit_exp(0, 2 * qt + 1)

            # ---- attention strips (one flat cross-strip pipeline) ----------
            psum_o = tc.alloc_tile_pool(name="psum_o", bufs=1, space="PSUM")
            for G in range(32):
                st, g = divmod(G, 8)
                emit_av(st, 2 * g)
                emit_av(st, 2 * g + 1)
                if g == 7:
                    emit_out(st)
                GE = G + 4
                if GE < 32:
                    ste, ge = divmod(GE, 8)
                    emit_e_group(ste, ge)
                    emit_exp(ste, 2 * ge + 1)
                    emit_exp(ste, 2 * ge)
            psum_o.release()
            psum_e.release()

    if not nc.is_finalized():
        nc.finalize()
    return nc


def kernel(x, Wq, bq, Wk, bk, Wv, bv, gamma):
    global last_results
    x = np.asarray(x, dtype=np.float32)
    Wq = np.asarray(Wq, dtype=np.float32)
    Wk = np.asarray(Wk, dtype=np.float32)
    Wv = np.asarray(Wv, dtype=np.float32)
    bq = np.asarray(bq, dtype=np.float32)
    bk = np.asarray(bk, dtype=np.float32)
    bv = np.asarray(bv, dtype=np.float32)
    gamma_v = float(np.asarray(gamma).reshape(-1)[0])
    assert x.shape == (B, C, H, W)

    has_bq = bool(np.any(bq != 0))
    has_bk = bool(np.any(bk != 0))
    has_bv = bool(np.any(bv != 0))

    key = (has_bq, has_bk, has_bv)
    if key not in _nc_cache:
        _nc_cache[key] = _build_nc(*key)
    nc = _nc_cache[key]

    f8 = ml_dtypes.float8_e4m3
    wqt4 = np.tile(Wq.T * S_W, (1, 4)).astype(f8)            # [C, 128]
    wkt4 = np.tile(Wk.T * S_W, (1, 4)).astype(f8)            # [C, 128]
    wvt = (Wv.T * (gamma_v * S_W)).astype(f8)                # [C, C]

    xf = x.reshape(B, C, N)
    in_maps = []
    for core in range(NCORES):
        b, half = divmod(core, 2)
        qsl = slice(half * MQ, (half + 1) * MQ)
        # rotate the core's query columns to the front; softmax over keys is
        # permutation-invariant so key order doesn't matter
        xrot = np.roll(xf[b], -half * MQ, axis=1) if half else xf[b]
        m = {
            "xb8": (xrot * S_X).astype(f8),
            "xqt": np.ascontiguousarray(xf[b][:, qsl].T).astype(np.float16),
            "wqt4": wqt4,
            "wkt4": wkt4,
            "wvt": wvt,
        }
        if has_bq:
            m["bq4"] = (np.tile(bq, 4) / 16.0).reshape(128, 1).astype(np.float32)
        if has_bk:
            m["bk4"] = np.tile(bk, 4).reshape(128, 1).astype(np.float32)
        if has_bv:
            m["bvg"] = np.broadcast_to(
                np.tile(bv * gamma_v, 2), (128, 512)).astype(np.float32).copy()
        in_maps.append(m)

    trace = bool(os.environ.get("BASS_TRACE"))
    if trace:
        try:
            import antenv.axon_hooks  # noqa: F401
        except ImportError:
            trace = False
    tmpdir = os.environ.get("BASS_KERNEL_TMPDIR") or None
    res = run_bass_kernel_spmd(nc, in_maps, list(range(NCORES)), trace=trace,
                               tmpdir=tmpdir)
    last_results = res

    out = np.empty((B, C, N), dtype=np.float32)
    for core in range(NCORES):
        b, half = divmod(core, 2)
        out[b, :, half * MQ:(half + 1) * MQ] = res.results[core]["out"].T.astype(np.float32)
    return out.reshape(B, C, H, W)
